# revision 102
# baseline (speedup 1.0000x reference)
"""MultiHeadLinearAttention Trainium2 kernel (8-core SPMD, fp8 DoubleRow).

Sharding: 16384 tokens split across 8 cores (core c: batch c//2, sequence half
c%2). All projections/attention/out-proj are local; the only cross-core
dependency is the per-batch KV summary (kv [H,DK,DK] + ksum [D]) reduced via a
266KB pair-wise AllReduce, overlapped with stage-2a q GLU compute.

Numerics: all seven big GEMMs (q/k/v GLU pairs + out-proj) run in fp8 e4m3
DoubleRow mode (host quantizes x*16 and W*16; PSUM carries 256*y with fp32
accumulation; attn is cast to fp8*16 on its PSUM eviction). DoubleRow
contracts two 128-row k-tiles per instruction at 0.5 cycles/row -- 4x fp32r
throughput. The attention summary (kv, ksum, z, num) stays bf16/fp32r.
End-to-end rel err vs the fp32 reference: 1.47e-2 (harness gate 2e-2).
The out-proj BIAS must stay exact (the output is the same magnitude as the
bias), so it enters PSUM as a fp32r K=1 outer product, not fp8.

Elementwise: GLU = 1 ACT Silu + 1 DVE stt (k/v bias enters PSUM via a
constant fp8 carrier k-tile pair; q bias via ACT per-partition bias/scale).
phi = elu(x)+1 computed exactly as min(exp(x), x+1): 1 ACT Exp + 1 DVE stt
(Exp overflow to +inf is absorbed by the min). The mask multiplies are folded
away: ksum takes the mask column as its matmul lhsT and vg's stt scalar slot
carries mask/256.

ACT table discipline: silu and exp live in different ACT table sets (1283ns
per reload), so stages are ordered k-GLU, v-GLU (all Silu), phi_k/ksum/kv
pass (all Exp), then stage 2 where the scheduler interleaves q-GLU (Silu)
chunks with the lagged phi_q/attention tail (Exp) at 2 reloads per chunk.

The reciprocal 1/(z+eps) is broadcast across each head's 64 partitions by one
matmul per head pair against a constant block "expander" E [16,128] (eps is
accumulated into the z PSUM as a K=1 outer product); num for both heads of a
pair comes from one full-array matmul against a block-diagonal kv tile.
"""
import os
from contextlib import ExitStack

import ml_dtypes
import numpy as np
import concourse.mybir as mybir
import concourse.tile as tile
from concourse import bacc
from concourse.bass_utils import run_bass_kernel_spmd

F32 = mybir.dt.float32
F32R = mybir.dt.float32r
F8 = mybir.dt.float8e4
BF16 = mybir.dt.bfloat16
ACTF = mybir.ActivationFunctionType
ALU = mybir.AluOpType
DR = mybir.MatmulPerfMode.DoubleRow

B, S, D, H = 4, 4096, 1024, 16
DK = D // H          # 64
EPS = 1e-6
NCORES = 8
T = B * S // NCORES  # 2048 tokens per core
P = 128
NM = T // P          # 16 token tiles
NCD = D // P         # 8 d-chunks
CH = 512             # stage-2 token chunk
NCH = T // CH        # 8 chunks
GROUPS = [[0, 1], [2, 3], [4, 5], [6, 7]]
SC = 256.0           # fp8 GEMM psum scale (sx*sw = 16*16)
LN_SC = float(np.log(SC))


def build(single_core=False):
    nc = bacc.Bacc("TRN2", target_bir_lowering=False, debug=False,
                   num_devices=1 if single_core else NCORES)
    dt_in = {}

    def inp(name, shape, dty=F32):
        dt_in[name] = nc.dram_tensor(name, shape, dty, kind="ExternalInput").ap()

    for name, shape in (
        ("xk8p", [P, NM * 8 * P]), ("xv8p", [P, NM * 8 * P]),
        ("xq8p", [P, NCH * 8 * CH]),
        ("wk18p", [P, 10 * D]), ("wk28p", [P, 10 * D]),
        ("wv18p", [P, 10 * D]), ("wv28p", [P, 10 * D]),
        ("wq18p", [P, 8 * D]), ("wq28p", [P, 8 * D]),
        ("wo8p", [P, 8 * D]), ("carrier", [P, 2 * P]),
    ):
        inp(name, shape, F8)
    for name, shape in (
        ("maskv", [P, NM]), ("bcq1", [P, NCD]), ("bcq2x", [P, NCD]),
        ("epsrow", [1, H]), ("onesrow", [1, CH]),
        ("zeros128", [P, P]), ("zeros512", [P, 4 * P]),
        ("eexp", [H, NCD * P]), ("borrow", [1, D]), ("ones128r", [1, P]),
    ):
        inp(name, shape, F32)
    inp("maskb", [P, NM], BF16)
    out = nc.dram_tensor("out", [T, D], F32, kind="ExternalOutput").ap()

    with tile.TileContext(nc) as tc:
        _emit(nc, tc, dt_in, out, single_core)
    nc.compile()
    return nc


def _emit(nc, tc, dt, out, single_core):
    def mm(psum, lhsT, rhs, start, stop):
        nc.tensor.matmul(psum, lhsT, rhs, start=start, stop=stop)

    def mm8(psum, lhsT, rhs, start, stop):
        nc.tensor.matmul(psum, lhsT, rhs, start=start, stop=stop, perf_mode=DR)

    with ExitStack() as st0:
        const = st0.enter_context(tc.tile_pool(name="const", bufs=1))
        dram = st0.enter_context(tc.tile_pool(name="dram", bufs=1, space="DRAM"))
        kvres = st0.enter_context(tc.tile_pool(name="kvres", bufs=1))
        kvstage_ctx = ExitStack()
        kvstage = kvstage_ctx.enter_context(tc.tile_pool(name="kvstage", bufs=1))

        carrier = const.tile([P, 2, P], F8, tag="carrier", name="carrier")
        nc.gpsimd.dma_start(carrier[:], dt["carrier"][:])
        maskb = const.tile([P, NM], BF16, tag="maskb", name="maskb")
        nc.gpsimd.dma_start(maskb[:], dt["maskb"][:])
        maskv = const.tile([P, NM], F32, tag="maskv", name="maskv")
        nc.gpsimd.dma_start(maskv[:], dt["maskv"][:])
        lnsc = const.tile([P, 1], F32, tag="lnsc", name="lnsc")
        nc.any.memset(lnsc[:], LN_SC)

        # ---- stage A/B/C pools (whole-stage-resident tiles) ----
        stab = st0.enter_context(ExitStack())
        kgp = stab.enter_context(tc.tile_pool(name="kgp", bufs=1))
        vgp = stab.enter_context(tc.tile_pool(name="vgp", bufs=1))
        phikp_ctx = ExitStack()
        phikp = phikp_ctx.enter_context(tc.tile_pool(name="phik", bufs=1))
        kg = [kgp.tile([P, D], BF16, tag=f"kg{m}", name=f"kg{m}")
              for m in range(NM)]
        vg = [vgp.tile([P, D], BF16, tag=f"vg{m}", name=f"vg{m}")
              for m in range(NM)]
        phi_k = [phikp.tile([P, D], BF16, tag=f"phik{m}", name=f"phik{m}")
                 for m in range(NM)]

        stxw = ExitStack()
        wkvp = stxw.enter_context(tc.tile_pool(name="wkv", bufs=1))
        xkvp = stxw.enter_context(tc.tile_pool(name="xkv", bufs=1))
        # startup order: first k-tile x quarter + first weight halves land
        # before the rest so m=0 GEMMs start ~5us in
        x_sb = {}
        x_sb["k"] = xkvp.tile([P, NM, 8, P], F8, tag="xk", name="xk")
        nc.sync.dma_start(x_sb["k"][:, 0:4, :, :], dt["xk8p"][:, 0:4096])
        w_sb = {}
        for w, src in (("k1", "wk18p"), ("k2", "wk28p")):
            w_sb[w] = wkvp.tile([P, 10, D], F8, tag=f"w{w}", name=f"w{w}")
            nc.sync.dma_start(
                w_sb[w][:, :, 0:512],
                dt[src][:].rearrange("p (kt o) -> p kt o", kt=10)[:, :, 0:512])
        for w, src in (("k1", "wk18p"), ("k2", "wk28p")):
            nc.sync.dma_start(
                w_sb[w][:, :, 512:D],
                dt[src][:].rearrange("p (kt o) -> p kt o", kt=10)[:, :, 512:D])
        for qtr in range(1, 4):
            nc.sync.dma_start(x_sb["k"][:, qtr * 4:(qtr + 1) * 4, :, :],
                              dt["xk8p"][:, qtr * 4096:(qtr + 1) * 4096])
        # prefetch v inputs (overlaps stage A compute)
        for w, src in (("v1", "wv18p"), ("v2", "wv28p")):
            w_sb[w] = wkvp.tile([P, 10, D], F8, tag=f"w{w}", name=f"w{w}")
            nc.gpsimd.dma_start(w_sb[w][:], dt[src][:])
        x_sb["v"] = xkvp.tile([P, NM, 8, P], F8, tag="xv", name="xv")
        for qtr in range(4):
            nc.gpsimd.dma_start(x_sb["v"][:, qtr * 4:(qtr + 1) * 4, :, :],
                                dt["xv8p"][:, qtr * 4096:(qtr + 1) * 4096])

        # ============ stages A (k GLU) and B (v GLU): all-Silu ============
        def glu_stage(which, out_tiles):
            with ExitStack() as stg:
                p1p = stg.enter_context(
                    tc.tile_pool(name=f"p1{which}", bufs=4, space="PSUM"))
                p2p = stg.enter_context(
                    tc.tile_pool(name=f"p2{which}", bufs=4, space="PSUM"))
                tp = stg.enter_context(tc.tile_pool(name=f"t{which}", bufs=3))
                w1 = w_sb[which + "1"]
                w2 = w_sb[which + "2"]
                xt = x_sb[which]
                for m in range(NM):
                    for n in range(2):
                        ns = slice(n * 512, (n + 1) * 512)
                        p1 = p1p.tile([P, 512], F32, tag="p1", name="p1")
                        p2 = p2p.tile([P, 512], F32, tag="p2", name="p2")
                        for k in range(4):
                            mm8(p1[:], xt[:, m, 2 * k:2 * k + 2, :],
                                w1[:, 2 * k:2 * k + 2, ns],
                                start=(k == 0), stop=False)
                        mm8(p1[:], carrier[:], w1[:, 8:10, ns],
                            start=False, stop=True)
                        for k in range(4):
                            mm8(p2[:], xt[:, m, 2 * k:2 * k + 2, :],
                                w2[:, 2 * k:2 * k + 2, ns],
                                start=(k == 0), stop=False)
                        mm8(p2[:], carrier[:], w2[:, 8:10, ns],
                            start=False, stop=True)
                        s1 = tp.tile([P, 512], F32, tag="s1", name="s1")
                        nc.scalar.activation(s1[:], p1[:], ACTF.Silu,
                                             scale=1.0 / SC)
                        if which == "k":
                            # kg = (p2/SC) * s1  (true scale)
                            nc.vector.scalar_tensor_tensor(
                                out_tiles[m][:, ns], p2[:], 1.0 / SC, s1[:],
                                ALU.mult, ALU.mult)
                        else:
                            # vg = (p2 * mask/SC) * s1  (mask folded in)
                            nc.vector.scalar_tensor_tensor(
                                out_tiles[m][:, ns], p2[:], maskv[:, m:m + 1],
                                s1[:], ALU.mult, ALU.mult)

        glu_stage("k", kg)
        glu_stage("v", vg)
        stxw.close()  # frees k/v weights + x tiles before stage-2 prefetch

        # prefetch stage-2 inputs (overlap stage C + collective)
        st2pre = st0.enter_context(ExitStack())
        wqp = st2pre.enter_context(tc.tile_pool(name="wqp", bufs=1, side="right"))
        wq1 = wqp.tile([P, 8, D], F8, tag="wq1", name="wq1")
        nc.sync.dma_start(wq1[:], dt["wq18p"][:])
        wq2 = wqp.tile([P, 8, D], F8, tag="wq2", name="wq2")
        nc.sync.dma_start(wq2[:], dt["wq28p"][:])
        xq8 = wqp.tile([P, NCH, 8, CH], F8, tag="xq8", name="xq8")
        for qtr in range(NCH):
            nc.sync.dma_start(
                xq8[:, qtr, :, :],
                dt["xq8p"][:, qtr * 8 * CH:(qtr + 1) * 8 * CH])

        # ====== stage C: phi_k (all-Exp) + ksum + kv accumulation ======
        with ExitStack() as stc:
            tcp = stc.enter_context(tc.tile_pool(name="tc", bufs=3))
            pksp = stc.enter_context(tc.tile_pool(name="pks", bufs=1, space="PSUM"))
            pkvp = stc.enter_context(tc.tile_pool(name="pkv", bufs=1, space="PSUM"))
            psum_ks = [pksp.tile([1, 512], F32, tag=f"ks{i}", name=f"ks{i}")
                       for i in range(2)]
            psum_kv = [pkvp.tile([64, 512], F32, tag=f"pkv{i}", name=f"pkv{i}")
                       for i in range(2)]

            def ksum_tail(m):
                for i in range(2):
                    mm(psum_ks[i][:], maskb[:, m:m + 1],
                       phi_k[m][:, i * 512:(i + 1) * 512],
                       start=(m == 0), stop=(m == NM - 1))

            def kv_tail(m):
                # one global accumulation group per bank: start only on the
                # very first matmul (has_written is per element)
                for h in range(H):
                    hs = slice(h * DK, (h + 1) * DK)
                    first = (m == 0 and h % 8 == 0)
                    last = (m == NM - 1 and h % 8 == 7)
                    nc.tensor.matmul(
                        psum_kv[h // 8][0:64, (h % 8) * DK:(h % 8 + 1) * DK],
                        phi_k[m][:, hs], vg[m][:, hs],
                        start=first, stop=last,
                        skip_group_check=not (first or last))

            for m in range(NM):
                texp = tcp.tile([P, D], F32, tag="texp", name="texp")
                nc.scalar.activation(texp[:], kg[m][:], ACTF.Exp)
                # phi_k = min(kg + 1, exp(kg)) = elu(kg) + 1
                nc.vector.scalar_tensor_tensor(
                    phi_k[m][:], kg[m][:], 1.0, texp[:], ALU.add, ALU.min)
                if m >= 2:
                    ksum_tail(m - 2)
                    kv_tail(m - 2)
            for m in (NM - 2, NM - 1):
                ksum_tail(m)
                kv_tail(m)
            cc_ks_sb = kvstage.tile([1, D], F32, tag="cc_ks_sb", name="cc_ks_sb")
            for i in range(2):
                nc.vector.tensor_copy(cc_ks_sb[0:1, i * 512:(i + 1) * 512],
                                      psum_ks[i][:])
            kv_acc = [kvstage.tile([64, 512], F32, tag=f"kv_acc{i}",
                                   name=f"kv_acc{i}") for i in range(2)]
            for i in range(2):
                nc.vector.tensor_copy(kv_acc[i][:], psum_kv[i][:])

        phikp_ctx.close()
        stab.close()     # frees kg/vg tiles

        # ============ collective: pair AllReduce of kv + ksum ============
        cc_in = dram.tile([130, 512], F32)
        cc_out = dram.tile([130, 512], F32)
        nc.gpsimd.dma_start(cc_in[0:64, :], kv_acc[0][:])
        nc.gpsimd.dma_start(cc_in[64:128, :], kv_acc[1][:])
        nc.gpsimd.dma_start(cc_in[128:130, :], cc_ks_sb[:])
        kvstage_ctx.close()
        if single_core:
            nc.sync.dma_start(cc_out[:], cc_in[:])
        else:
            nc.gpsimd.collective_compute(
                "AllReduce", ALU.add, replica_groups=GROUPS,
                ins=[cc_in.opt()], outs=[cc_out.opt()])

        # reduced kv -> per-pair block-diag lhsT slices of one tile (a single
        # full-array matmul computes both heads of a pair; avoids PE
        # quadrant-3 subtiling). All rebuild DMAs go on the idle SP engine.
        kv_bd_t = kvres.tile([P, NCD, P], F32R, tag="kvbd", name="kvbd")
        for i in range(2):
            nc.sync.dma_start(kv_bd_t[:, 4 * i:4 * i + 4, :],
                              dt["zeros512"][:].bitcast(F32R))
        for pair in range(NCD):
            for i, h in ((0, 2 * pair), (64, 2 * pair + 1)):
                r0 = 0 if h < 8 else 64
                nc.sync.dma_start(
                    kv_bd_t[i:i + 64, pair, i:i + 64],
                    cc_out[r0:r0 + 64,
                           (h % 8) * DK:(h % 8 + 1) * DK].bitcast(F32R))
        kv_bd = [kv_bd_t[:, pair, :] for pair in range(NCD)]
        ksum_bd_t = kvres.tile([P, NCD, H], F32R, tag="ksbd", name="ksbd")
        nc.sync.dma_start(ksum_bd_t[:, :, :],
                          dt["zeros128"][:, 0:NCD * H].bitcast(F32R))
        for c in range(NCD):
            # ksum[d] lives at cc_out[128 + d // 512, d % 512]
            for half, cs in ((0, 2 * c), (64, 2 * c + 1)):
                d0 = c * P + half
                nc.sync.dma_start(
                    ksum_bd_t[half:half + 64, c, cs:cs + 1],
                    cc_out[128 + d0 // 512:129 + d0 // 512,
                           d0 % 512:d0 % 512 + 64].bitcast(F32R))
        ksum_bd = [ksum_bd_t[:, c, :] for c in range(NCD)]

        # ============ stage 2a: q GLU (all-Silu) -> qg (256x scale) ============
        bcq1 = const.tile([P, NCD], F32, tag="bcq1", name="bcq1")
        nc.gpsimd.dma_start(bcq1[:], dt["bcq1"][:])
        bcq2x = const.tile([P, NCD], F32, tag="bcq2x", name="bcq2x")
        nc.gpsimd.dma_start(bcq2x[:], dt["bcq2x"][:])

        qgp_ctx = ExitStack()
        qgp = qgp_ctx.enter_context(tc.tile_pool(name="qgp", bufs=1))
        qg = [qgp.tile([P, NCH, CH], BF16, tag=f"qg{mc}", name=f"qg{mc}")
              for mc in range(NCD)]
        with ExitStack() as st2a:
            pq1p = st2a.enter_context(tc.tile_pool(name="pq1", bufs=4, space="PSUM"))
            pq2p = st2a.enter_context(tc.tile_pool(name="pq2", bufs=4, space="PSUM"))
            t2a = st2a.enter_context(tc.tile_pool(name="t2a", bufs=3))
            for ch in range(NCH):
                for mc in range(NCD):
                    ms = slice(mc * P, (mc + 1) * P)
                    p1 = pq1p.tile([P, CH], F32, tag="pq1", name="pq1")
                    p2 = pq2p.tile([P, CH], F32, tag="pq2", name="pq2")
                    for k in range(4):
                        mm8(p1[:], wq1[:, 2 * k:2 * k + 2, ms],
                            xq8[:, ch, 2 * k:2 * k + 2, :],
                            start=(k == 0), stop=(k == 3))
                    for k in range(4):
                        mm8(p2[:], wq2[:, 2 * k:2 * k + 2, ms],
                            xq8[:, ch, 2 * k:2 * k + 2, :],
                            start=(k == 0), stop=(k == 3))
                    s1 = t2a.tile([P, CH], F32, tag="qs1", name="qs1")
                    nc.scalar.activation(s1[:], p1[:], ACTF.Silu,
                                         scale=1.0 / SC,
                                         bias=bcq1[:, mc:mc + 1])
                    # qg' = (p2 + 256*b2) * s1 = 256*g
                    nc.vector.scalar_tensor_tensor(
                        qg[mc][:, ch, :], p2[:], bcq2x[:, mc:mc + 1], s1[:],
                        ALU.add, ALU.mult)

        # ===== stage 2b: phi_q (all-Exp) + attention tail + out-proj =====
        with ExitStack() as st2b:
            wop = st2b.enter_context(tc.tile_pool(name="wo", bufs=1))
            phiqp = st2b.enter_context(tc.tile_pool(name="phiq", bufs=3))
            attnp = st2b.enter_context(tc.tile_pool(name="attn", bufs=3))
            t2b = st2b.enter_context(tc.tile_pool(name="t2b", bufs=4))
            tzp = st2b.enter_context(tc.tile_pool(name="tz", bufs=3))
            osbp = st2b.enter_context(tc.tile_pool(name="osb", bufs=3))
            pzp = st2b.enter_context(tc.tile_pool(name="pz", bufs=2, space="PSUM"))
            prp = st2b.enter_context(tc.tile_pool(name="pr", bufs=2, space="PSUM"))
            pnp = st2b.enter_context(tc.tile_pool(name="pn", bufs=2, space="PSUM"))
            pop = st2b.enter_context(tc.tile_pool(name="po", bufs=2, space="PSUM"))

            e_sb = wop.tile([H, NCD * P], F32R, tag="eexp", name="eexp")
            nc.scalar.dma_start(e_sb[:], dt["eexp"][:].bitcast(F32R))
            wo8 = wop.tile([P, 8, D], F8, tag="wo8", name="wo8")
            nc.scalar.dma_start(wo8[:], dt["wo8p"][:])
            borrow = wop.tile([1, D], F32R, tag="borrow", name="borrow")
            nc.scalar.dma_start(borrow[:], dt["borrow"][:].bitcast(F32R))
            ones128r = wop.tile([1, P], F32R, tag="ones128r", name="ones128r")
            nc.scalar.dma_start(ones128r[:], dt["ones128r"][:].bitcast(F32R))
            epse = wop.tile([1, H], F32R, tag="epse", name="epse")
            nc.scalar.dma_start(epse[:], dt["epsrow"][:].bitcast(F32R))
            onesr = wop.tile([1, CH], F32R, tag="onesr", name="onesr")
            nc.scalar.dma_start(onesr[:], dt["onesrow"][:].bitcast(F32R))

            def phi_pass(ch):
                phi_q = [phiqp.tile([P, CH], F32R, tag=f"phiq{mc}",
                                    name=f"phiq{mc}") for mc in range(NCD)]
                for mc in range(NCD):
                    texp = t2b.tile([P, CH], F32, tag="qtexp", name="qtexp")
                    # 256*exp(g) = exp(qg'/256 + ln 256)
                    nc.scalar.activation(texp[:], qg[mc][:, ch, :], ACTF.Exp,
                                         scale=1.0 / SC, bias=lnsc[:])
                    # phi' = min(qg' + 256, 256*exp(g)) = 256*(elu(g)+1)
                    nc.vector.scalar_tensor_tensor(
                        phi_q[mc][:], qg[mc][:, ch, :], SC, texp[:],
                        ALU.add, ALU.min)
                return phi_q

            def tail_head(phi_q):
                pz = pzp.tile([H, CH], F32, tag="pz", name="pz")
                for c in range(NCD):
                    mm(pz[:], ksum_bd[c][:], phi_q[c][:],
                       start=(c == 0), stop=False)
                # eps lands in PSUM via a K=1 outer product: eps_col x ones
                mm(pz[:], epse[:], onesr[:], start=False, stop=True)
                r_sb = tzp.tile([H, CH], F32R, tag="r_sb", name="r_sb")
                with nc.allow_low_precision(reason="fp32r feeds r-broadcast mm"):
                    nc.vector.reciprocal(r_sb[:], pz[:])
                attn8 = attnp.tile([P, NCD, CH], F8, tag="attn8", name="attn8")
                return r_sb, attn8

            def tail_pair(phi_q, r_sb, attn8, pair):
                # r broadcast across each head's 64 partitions via expander E
                pr2 = prp.tile([P, CH], F32, tag="pr", name="pr")
                mm(pr2[:], e_sb[:, pair * P:(pair + 1) * P], r_sb[:],
                   start=True, stop=True)
                r_rep = t2b.tile([P, CH], F32, tag="r_rep", name="r_rep")
                if pair % 4 != 3:
                    nc.scalar.activation(r_rep[:], pr2[:], ACTF.Copy)
                else:
                    nc.vector.tensor_copy(r_rep[:], pr2[:])
                pn2 = pnp.tile([P, CH], F32, tag="pn", name="pn")
                mm(pn2[:], kv_bd[pair][:], phi_q[pair][:], start=True, stop=True)
                # attn8 = fp8(16 * num / (z+eps)) feeds the fp8 out-proj
                nc.vector.scalar_tensor_tensor(attn8[:, pair, :], pn2[:], 16.0,
                                               r_rep[:], ALU.mult, ALU.mult)

            def tail_out(ch, attn8):
                for mt in range(CH // P):
                    o_sb = osbp.tile([P, D], F32, tag="o_sb", name="o_sb")
                    for n in range(2):
                        ns = slice(n * 512, (n + 1) * 512)
                        po = pop.tile([P, 512], F32, tag="po", name="po")
                        for k in range(4):
                            mm8(po[:], attn8[:, 2 * k:2 * k + 2,
                                             mt * P:(mt + 1) * P],
                                wo8[:, 2 * k:2 * k + 2, ns],
                                start=(k == 0), stop=False)
                        # exact bias enters PSUM as a fp32r K=1 outer product
                        # (the output is the same magnitude as the bias, so
                        # the bias cannot be fp8-quantized)
                        mm(po[:], ones128r[:], borrow[:, ns],
                           start=False, stop=True)
                        nc.scalar.activation(o_sb[:, ns], po[:], ACTF.Copy,
                                             scale=1.0 / SC)
                    row0 = ch * CH + mt * P
                    nc.sync.dma_start(out[row0:row0 + P, :], o_sb[:])

            def run_tail(p_ch, p_phi):
                p_r, p_attn = tail_head(p_phi)
                for pair in range(NCD):
                    tail_pair(p_phi, p_r, p_attn, pair)
                tail_out(p_ch, p_attn)

            pending = []
            for ch in range(NCH):
                pending.append((ch, phi_pass(ch)))
                if len(pending) > 1:
                    run_tail(*pending.pop(0))
            for item in pending:
                run_tail(*item)
        qgp_ctx.close()


_CACHE = {}


def _get_nc(single_core=False):
    key = bool(single_core)
    if key not in _CACHE:
        _CACHE[key] = build(single_core)
    return _CACHE[key]


def _q8(a):
    return a.astype(ml_dtypes.float8_e4m3)


def _pack_x(xT, nt, tw):
    # xT [D, T] fp8 -> [p, tile, kt, t] -> [P, nt*8*tw]
    return np.ascontiguousarray(
        xT.reshape(8, P, nt, tw).transpose(1, 2, 0, 3).reshape(P, nt * 8 * tw))


def _pack_w(wT, b=None):
    # wT [D, D] fp32 -> fp8 [p, kt, o] (+ optional bias carrier rows kt=8,9)
    w8 = _q8(wT * 16.0).reshape(8, P, D).transpose(1, 0, 2)
    if b is None:
        return np.ascontiguousarray(w8.reshape(P, 8 * D))
    blk = np.zeros((P, 2, D), ml_dtypes.float8_e4m3)
    blk[0, 0, :] = _q8(b * 16.0)
    return np.ascontiguousarray(
        np.concatenate([w8, blk], axis=1).reshape(P, 10 * D))


def _pack_w_hilo(wT):
    # wT [D, D] fp32 -> fp8 hi (kt 0-7) + lo residual (kt 8-15), one scale
    w16 = wT.astype(np.float32) * 16.0
    hi = _q8(w16)
    lo = _q8(w16 - hi.astype(np.float32))
    pk = lambda a: a.reshape(8, P, D).transpose(1, 0, 2)
    return np.ascontiguousarray(
        np.concatenate([pk(hi), pk(lo)], axis=1).reshape(P, 16 * D))


def make_in_maps(inputs):
    f = np.float32
    q = np.asarray(inputs["query"], f).reshape(B * S, D)
    k = np.asarray(inputs["key"], f).reshape(B * S, D)
    v = np.asarray(inputs["value"], f).reshape(B * S, D)
    mask = np.asarray(inputs["mask"], f).reshape(B * S)

    carrier = np.zeros((P, 2, P), ml_dtypes.float8_e4m3)
    carrier[0, 0, :] = ml_dtypes.float8_e4m3(16.0)
    eexp = np.zeros((H, NCD * P), f)
    for pair in range(NCD):
        eexp[2 * pair, pair * P:pair * P + 64] = 1.0
        eexp[2 * pair + 1, pair * P + 64:(pair + 1) * P] = 1.0

    shared = {
        "wk18p": _pack_w(np.asarray(inputs["k_w1"], f).T,
                         np.asarray(inputs["k_b1"], f)),
        "wk28p": _pack_w(np.asarray(inputs["k_w2"], f).T,
                         np.asarray(inputs["k_b2"], f)),
        "wv18p": _pack_w(np.asarray(inputs["v_w1"], f).T,
                         np.asarray(inputs["v_b1"], f)),
        "wv28p": _pack_w(np.asarray(inputs["v_w2"], f).T,
                         np.asarray(inputs["v_b2"], f)),
        "wq18p": _pack_w(np.asarray(inputs["q_w1"], f).T),
        "wq28p": _pack_w(np.asarray(inputs["q_w2"], f).T),
        "wo8p": _pack_w(np.asarray(inputs["out_w"], f).T),
        "borrow": np.asarray(inputs["out_b"], f).reshape(1, D) * SC,
        "ones128r": np.ones((1, P), f),
        "bcq1": np.ascontiguousarray(np.asarray(inputs["q_b1"], f).reshape(NCD, P).T),
        "bcq2x": np.ascontiguousarray(
            (np.asarray(inputs["q_b2"], f) * SC).reshape(NCD, P).T),
        "zeros128": np.zeros((P, P), f),
        "zeros512": np.zeros((P, 4 * P), f),
        "epsrow": np.full((1, H), SC * EPS, f),
        "onesrow": np.ones((1, CH), f),
        "carrier": np.ascontiguousarray(carrier.reshape(P, 2 * P)),
        "eexp": eexp,
    }
    in_maps = []
    for c in range(NCORES):
        sl = slice(c * T, (c + 1) * T)
        m = dict(shared)
        m["xq8p"] = _pack_x(_q8(q[sl].T * 16.0), NCH, CH)
        m["xk8p"] = _pack_x(_q8(k[sl].T * 16.0), NM, P)
        m["xv8p"] = _pack_x(_q8(v[sl].T * 16.0), NM, P)
        mcol = np.ascontiguousarray(mask[sl].reshape(NM, P).T)
        m["maskb"] = mcol.astype(ml_dtypes.bfloat16)
        m["maskv"] = np.ascontiguousarray(mcol / SC)
        in_maps.append(m)
    return in_maps


def kernel(**inputs):
    nc = _get_nc(False)
    in_maps = make_in_maps(inputs)
    res = run_bass_kernel_spmd(nc, in_maps, list(range(NCORES))).results
    outc = np.concatenate([res[c]["out"] for c in range(NCORES)], axis=0)
    return outc.reshape(B, S, D)


# revision 103
# speedup vs baseline: 1.0062x; 1.0062x over previous
"""MultiHeadLinearAttention Trainium2 kernel (8-core SPMD, fp8 DoubleRow).

Sharding: 16384 tokens split across 8 cores (core c: batch c//2, sequence half
c%2). All projections/attention/out-proj are local; the only cross-core
dependency is the per-batch KV summary (kv [H,DK,DK] + ksum [D]) reduced via a
266KB pair-wise AllReduce, overlapped with stage-2a q GLU compute.

Numerics: all seven big GEMMs (q/k/v GLU pairs + out-proj) run in fp8 e4m3
DoubleRow mode (host quantizes x*16 and W*16; PSUM carries 256*y with fp32
accumulation; attn is cast to fp8*16 on its PSUM eviction). DoubleRow
contracts two 128-row k-tiles per instruction at 0.5 cycles/row -- 4x fp32r
throughput. The attention summary (kv, ksum, z, num) stays bf16/fp32r.
End-to-end rel err vs the fp32 reference: 1.47e-2 (harness gate 2e-2).
The out-proj BIAS must stay exact (the output is the same magnitude as the
bias), so it enters PSUM as a fp32r K=1 outer product, not fp8.

Elementwise: GLU = 1 ACT Silu + 1 DVE stt (k/v bias enters PSUM via a
constant fp8 carrier k-tile pair; q bias via ACT per-partition bias/scale).
phi = elu(x)+1 computed exactly as min(exp(x), x+1): 1 ACT Exp + 1 DVE stt
(Exp overflow to +inf is absorbed by the min). The mask multiplies are folded
away: ksum takes the mask column as its matmul lhsT and vg's stt scalar slot
carries mask/256.

ACT table discipline: silu and exp live in different ACT table sets (1283ns
per reload), so stages are ordered k-GLU, v-GLU (all Silu), phi_k/ksum/kv
pass (all Exp), then stage 2 where the scheduler interleaves q-GLU (Silu)
chunks with the lagged phi_q/attention tail (Exp) at 2 reloads per chunk.

The reciprocal 1/(z+eps) is broadcast across each head's 64 partitions by one
matmul per head pair against a constant block "expander" E [16,128] (eps is
accumulated into the z PSUM as a K=1 outer product); num for both heads of a
pair comes from one full-array matmul against a block-diagonal kv tile.
"""
import os
from contextlib import ExitStack

import ml_dtypes
import numpy as np
import concourse.mybir as mybir
import concourse.tile as tile
from concourse import bacc
from concourse.bass_utils import run_bass_kernel_spmd

F32 = mybir.dt.float32
F32R = mybir.dt.float32r
F8 = mybir.dt.float8e4
BF16 = mybir.dt.bfloat16
ACTF = mybir.ActivationFunctionType
ALU = mybir.AluOpType
DR = mybir.MatmulPerfMode.DoubleRow

B, S, D, H = 4, 4096, 1024, 16
DK = D // H          # 64
EPS = 1e-6
NCORES = 8
T = B * S // NCORES  # 2048 tokens per core
P = 128
NM = T // P          # 16 token tiles
NCD = D // P         # 8 d-chunks
CH = 512             # stage-2 token chunk
NCH = T // CH        # 8 chunks
GROUPS = [[0, 1], [2, 3], [4, 5], [6, 7]]
SC = 256.0           # fp8 GEMM psum scale (sx*sw = 16*16)
LN_SC = float(np.log(SC))


def build(single_core=False):
    nc = bacc.Bacc("TRN2", target_bir_lowering=False, debug=False,
                   num_devices=1 if single_core else NCORES)
    dt_in = {}

    def inp(name, shape, dty=F32):
        dt_in[name] = nc.dram_tensor(name, shape, dty, kind="ExternalInput").ap()

    for name, shape in (
        ("xk8p", [P, NM * 8 * P]), ("xv8p", [P, NM * 8 * P]),
        ("xq8p", [P, NCH * 8 * CH]),
        ("wk18p", [P, 10 * D]), ("wk28p", [P, 10 * D]),
        ("wv18p", [P, 10 * D]), ("wv28p", [P, 10 * D]),
        ("wq18p", [P, 8 * D]), ("wq28p", [P, 8 * D]),
        ("wo8p", [P, 8 * D]), ("carrier", [P, 2 * P]),
    ):
        inp(name, shape, F8)
    for name, shape in (
        ("maskv", [P, NM]), ("bcq1", [P, NCD]), ("bcq2x", [P, NCD]),
        ("epsrow", [1, H]), ("onesrow", [1, CH]),
        ("zeros128", [P, P]), ("zeros512", [P, 4 * P]),
        ("eexp", [H, NCD * P]), ("borrow", [1, D]), ("ones128r", [1, P]),
    ):
        inp(name, shape, F32)
    inp("maskb", [P, NM], BF16)
    out = nc.dram_tensor("out", [T, D], F32, kind="ExternalOutput").ap()

    with tile.TileContext(nc) as tc:
        _emit(nc, tc, dt_in, out, single_core)
    nc.compile()
    return nc


def _emit(nc, tc, dt, out, single_core):
    def mm(psum, lhsT, rhs, start, stop):
        nc.tensor.matmul(psum, lhsT, rhs, start=start, stop=stop)

    def mm8(psum, lhsT, rhs, start, stop):
        nc.tensor.matmul(psum, lhsT, rhs, start=start, stop=stop, perf_mode=DR)

    with ExitStack() as st0:
        const = st0.enter_context(tc.tile_pool(name="const", bufs=1))
        dram = st0.enter_context(tc.tile_pool(name="dram", bufs=1, space="DRAM"))
        kvres = st0.enter_context(tc.tile_pool(name="kvres", bufs=1))
        kvstage_ctx = ExitStack()
        kvstage = kvstage_ctx.enter_context(tc.tile_pool(name="kvstage", bufs=1))

        carrier = const.tile([P, 2, P], F8, tag="carrier", name="carrier")
        nc.gpsimd.dma_start(carrier[:], dt["carrier"][:])
        maskb = const.tile([P, NM], BF16, tag="maskb", name="maskb")
        nc.gpsimd.dma_start(maskb[:], dt["maskb"][:])
        maskv = const.tile([P, NM], F32, tag="maskv", name="maskv")
        nc.gpsimd.dma_start(maskv[:], dt["maskv"][:])
        lnsc = const.tile([P, 1], F32, tag="lnsc", name="lnsc")
        nc.any.memset(lnsc[:], LN_SC)

        # ---- stage A/B/C pools (whole-stage-resident tiles) ----
        stab = st0.enter_context(ExitStack())
        kgp = stab.enter_context(tc.tile_pool(name="kgp", bufs=1))
        vgp = stab.enter_context(tc.tile_pool(name="vgp", bufs=1))
        phikp_ctx = ExitStack()
        phikp = phikp_ctx.enter_context(tc.tile_pool(name="phik", bufs=1))
        kg = [kgp.tile([P, D], BF16, tag=f"kg{m}", name=f"kg{m}")
              for m in range(NM)]
        vg = [vgp.tile([P, D], BF16, tag=f"vg{m}", name=f"vg{m}")
              for m in range(NM)]
        phi_k = [phikp.tile([P, D], BF16, tag=f"phik{m}", name=f"phik{m}")
                 for m in range(NM)]

        stxw = ExitStack()
        wkvp = stxw.enter_context(tc.tile_pool(name="wkv", bufs=1))
        xkvp = stxw.enter_context(tc.tile_pool(name="xkv", bufs=1))
        # startup order: first k-tile x quarter + first weight halves land
        # before the rest so m=0 GEMMs start ~5us in
        x_sb = {}
        x_sb["k"] = xkvp.tile([P, NM, 8, P], F8, tag="xk", name="xk")
        nc.sync.dma_start(x_sb["k"][:, 0:4, :, :], dt["xk8p"][:, 0:4096])
        w_sb = {}
        for w, src in (("k1", "wk18p"), ("k2", "wk28p")):
            w_sb[w] = wkvp.tile([P, 10, D], F8, tag=f"w{w}", name=f"w{w}")
            nc.sync.dma_start(
                w_sb[w][:, :, 0:512],
                dt[src][:].rearrange("p (kt o) -> p kt o", kt=10)[:, :, 0:512])
        for w, src in (("k1", "wk18p"), ("k2", "wk28p")):
            nc.sync.dma_start(
                w_sb[w][:, :, 512:D],
                dt[src][:].rearrange("p (kt o) -> p kt o", kt=10)[:, :, 512:D])
        for qtr in range(1, 4):
            nc.sync.dma_start(x_sb["k"][:, qtr * 4:(qtr + 1) * 4, :, :],
                              dt["xk8p"][:, qtr * 4096:(qtr + 1) * 4096])
        # prefetch v inputs (overlaps stage A compute)
        for w, src in (("v1", "wv18p"), ("v2", "wv28p")):
            w_sb[w] = wkvp.tile([P, 10, D], F8, tag=f"w{w}", name=f"w{w}")
            nc.gpsimd.dma_start(w_sb[w][:], dt[src][:])
        x_sb["v"] = xkvp.tile([P, NM, 8, P], F8, tag="xv", name="xv")
        for qtr in range(4):
            nc.gpsimd.dma_start(x_sb["v"][:, qtr * 4:(qtr + 1) * 4, :, :],
                                dt["xv8p"][:, qtr * 4096:(qtr + 1) * 4096])

        # ============ stages A (k GLU) and B (v GLU): all-Silu ============
        def glu_stage(which, out_tiles):
            with ExitStack() as stg:
                p1p = stg.enter_context(
                    tc.tile_pool(name=f"p1{which}", bufs=4, space="PSUM"))
                p2p = stg.enter_context(
                    tc.tile_pool(name=f"p2{which}", bufs=4, space="PSUM"))
                tp = stg.enter_context(tc.tile_pool(name=f"t{which}", bufs=6))
                w1 = w_sb[which + "1"]
                w2 = w_sb[which + "2"]
                xt = x_sb[which]
                for m in range(NM):
                    for n in range(2):
                        ns = slice(n * 512, (n + 1) * 512)
                        p1 = p1p.tile([P, 512], F32, tag="p1", name="p1")
                        p2 = p2p.tile([P, 512], F32, tag="p2", name="p2")
                        for k in range(4):
                            mm8(p1[:], xt[:, m, 2 * k:2 * k + 2, :],
                                w1[:, 2 * k:2 * k + 2, ns],
                                start=(k == 0), stop=False)
                        mm8(p1[:], carrier[:], w1[:, 8:10, ns],
                            start=False, stop=True)
                        for k in range(4):
                            mm8(p2[:], xt[:, m, 2 * k:2 * k + 2, :],
                                w2[:, 2 * k:2 * k + 2, ns],
                                start=(k == 0), stop=False)
                        mm8(p2[:], carrier[:], w2[:, 8:10, ns],
                            start=False, stop=True)
                        s1 = tp.tile([P, 512], F32, tag="s1", name="s1")
                        nc.scalar.activation(s1[:], p1[:], ACTF.Silu,
                                             scale=1.0 / SC)
                        if which == "k":
                            # kg = (p2/SC) * s1  (true scale)
                            nc.vector.scalar_tensor_tensor(
                                out_tiles[m][:, ns], p2[:], 1.0 / SC, s1[:],
                                ALU.mult, ALU.mult)
                        else:
                            # vg = (p2 * mask/SC) * s1  (mask folded in)
                            nc.vector.scalar_tensor_tensor(
                                out_tiles[m][:, ns], p2[:], maskv[:, m:m + 1],
                                s1[:], ALU.mult, ALU.mult)

        glu_stage("k", kg)
        glu_stage("v", vg)
        stxw.close()  # frees k/v weights + x tiles before stage-2 prefetch

        # prefetch stage-2 inputs (overlap stage C + collective)
        st2pre = st0.enter_context(ExitStack())
        wqp = st2pre.enter_context(tc.tile_pool(name="wqp", bufs=1, side="right"))
        wq1 = wqp.tile([P, 8, D], F8, tag="wq1", name="wq1")
        nc.sync.dma_start(wq1[:], dt["wq18p"][:])
        wq2 = wqp.tile([P, 8, D], F8, tag="wq2", name="wq2")
        nc.sync.dma_start(wq2[:], dt["wq28p"][:])
        xq8 = wqp.tile([P, NCH, 8, CH], F8, tag="xq8", name="xq8")
        for qtr in range(NCH):
            nc.sync.dma_start(
                xq8[:, qtr, :, :],
                dt["xq8p"][:, qtr * 8 * CH:(qtr + 1) * 8 * CH])

        # ====== stage C: phi_k (all-Exp) + ksum + kv accumulation ======
        with ExitStack() as stc:
            tcp = stc.enter_context(tc.tile_pool(name="tc", bufs=6))
            pksp = stc.enter_context(tc.tile_pool(name="pks", bufs=1, space="PSUM"))
            pkvp = stc.enter_context(tc.tile_pool(name="pkv", bufs=1, space="PSUM"))
            psum_ks = [pksp.tile([1, 512], F32, tag=f"ks{i}", name=f"ks{i}")
                       for i in range(2)]
            psum_kv = [pkvp.tile([64, 512], F32, tag=f"pkv{i}", name=f"pkv{i}")
                       for i in range(2)]

            def ksum_tail(m):
                for i in range(2):
                    mm(psum_ks[i][:], maskb[:, m:m + 1],
                       phi_k[m][:, i * 512:(i + 1) * 512],
                       start=(m == 0), stop=(m == NM - 1))

            def kv_tail(m):
                # one global accumulation group per bank: start only on the
                # very first matmul (has_written is per element)
                for h in range(H):
                    hs = slice(h * DK, (h + 1) * DK)
                    first = (m == 0 and h % 8 == 0)
                    last = (m == NM - 1 and h % 8 == 7)
                    nc.tensor.matmul(
                        psum_kv[h // 8][0:64, (h % 8) * DK:(h % 8 + 1) * DK],
                        phi_k[m][:, hs], vg[m][:, hs],
                        start=first, stop=last,
                        skip_group_check=not (first or last))

            for m in range(NM):
                texp = tcp.tile([P, D], F32, tag="texp", name="texp")
                nc.scalar.activation(texp[:], kg[m][:], ACTF.Exp)
                # phi_k = min(kg + 1, exp(kg)) = elu(kg) + 1
                nc.vector.scalar_tensor_tensor(
                    phi_k[m][:], kg[m][:], 1.0, texp[:], ALU.add, ALU.min)
                if m >= 2:
                    ksum_tail(m - 2)
                    kv_tail(m - 2)
            for m in (NM - 2, NM - 1):
                ksum_tail(m)
                kv_tail(m)
            cc_ks_sb = kvstage.tile([1, D], F32, tag="cc_ks_sb", name="cc_ks_sb")
            for i in range(2):
                nc.vector.tensor_copy(cc_ks_sb[0:1, i * 512:(i + 1) * 512],
                                      psum_ks[i][:])
            kv_acc = [kvstage.tile([64, 512], F32, tag=f"kv_acc{i}",
                                   name=f"kv_acc{i}") for i in range(2)]
            for i in range(2):
                nc.vector.tensor_copy(kv_acc[i][:], psum_kv[i][:])

        phikp_ctx.close()
        stab.close()     # frees kg/vg tiles

        # ============ collective: pair AllReduce of kv + ksum ============
        cc_in = dram.tile([130, 512], F32)
        cc_out = dram.tile([130, 512], F32)
        nc.gpsimd.dma_start(cc_in[0:64, :], kv_acc[0][:])
        nc.gpsimd.dma_start(cc_in[64:128, :], kv_acc[1][:])
        nc.gpsimd.dma_start(cc_in[128:130, :], cc_ks_sb[:])
        kvstage_ctx.close()
        if single_core:
            nc.sync.dma_start(cc_out[:], cc_in[:])
        else:
            nc.gpsimd.collective_compute(
                "AllReduce", ALU.add, replica_groups=GROUPS,
                ins=[cc_in.opt()], outs=[cc_out.opt()])

        # reduced kv -> per-pair block-diag lhsT slices of one tile (a single
        # full-array matmul computes both heads of a pair; avoids PE
        # quadrant-3 subtiling). All rebuild DMAs go on the idle SP engine.
        kv_bd_t = kvres.tile([P, NCD, P], F32R, tag="kvbd", name="kvbd")
        for i in range(2):
            nc.sync.dma_start(kv_bd_t[:, 4 * i:4 * i + 4, :],
                              dt["zeros512"][:].bitcast(F32R))
        for pair in range(NCD):
            for i, h in ((0, 2 * pair), (64, 2 * pair + 1)):
                r0 = 0 if h < 8 else 64
                nc.sync.dma_start(
                    kv_bd_t[i:i + 64, pair, i:i + 64],
                    cc_out[r0:r0 + 64,
                           (h % 8) * DK:(h % 8 + 1) * DK].bitcast(F32R))
        kv_bd = [kv_bd_t[:, pair, :] for pair in range(NCD)]
        ksum_bd_t = kvres.tile([P, NCD, H], F32R, tag="ksbd", name="ksbd")
        nc.sync.dma_start(ksum_bd_t[:, :, :],
                          dt["zeros128"][:, 0:NCD * H].bitcast(F32R))
        for c in range(NCD):
            # ksum[d] lives at cc_out[128 + d // 512, d % 512]
            for half, cs in ((0, 2 * c), (64, 2 * c + 1)):
                d0 = c * P + half
                nc.sync.dma_start(
                    ksum_bd_t[half:half + 64, c, cs:cs + 1],
                    cc_out[128 + d0 // 512:129 + d0 // 512,
                           d0 % 512:d0 % 512 + 64].bitcast(F32R))
        ksum_bd = [ksum_bd_t[:, c, :] for c in range(NCD)]

        # ============ stage 2a: q GLU (all-Silu) -> qg (256x scale) ============
        bcq1 = const.tile([P, NCD], F32, tag="bcq1", name="bcq1")
        nc.gpsimd.dma_start(bcq1[:], dt["bcq1"][:])
        bcq2x = const.tile([P, NCD], F32, tag="bcq2x", name="bcq2x")
        nc.gpsimd.dma_start(bcq2x[:], dt["bcq2x"][:])

        qgp_ctx = ExitStack()
        qgp = qgp_ctx.enter_context(tc.tile_pool(name="qgp", bufs=1))
        qg = [qgp.tile([P, NCH, CH], BF16, tag=f"qg{mc}", name=f"qg{mc}")
              for mc in range(NCD)]
        with ExitStack() as st2a:
            pq1p = st2a.enter_context(tc.tile_pool(name="pq1", bufs=4, space="PSUM"))
            pq2p = st2a.enter_context(tc.tile_pool(name="pq2", bufs=4, space="PSUM"))
            t2a = st2a.enter_context(tc.tile_pool(name="t2a", bufs=6))
            for ch in range(NCH):
                for mc in range(NCD):
                    ms = slice(mc * P, (mc + 1) * P)
                    p1 = pq1p.tile([P, CH], F32, tag="pq1", name="pq1")
                    p2 = pq2p.tile([P, CH], F32, tag="pq2", name="pq2")
                    for k in range(4):
                        mm8(p1[:], wq1[:, 2 * k:2 * k + 2, ms],
                            xq8[:, ch, 2 * k:2 * k + 2, :],
                            start=(k == 0), stop=(k == 3))
                    for k in range(4):
                        mm8(p2[:], wq2[:, 2 * k:2 * k + 2, ms],
                            xq8[:, ch, 2 * k:2 * k + 2, :],
                            start=(k == 0), stop=(k == 3))
                    s1 = t2a.tile([P, CH], F32, tag="qs1", name="qs1")
                    nc.scalar.activation(s1[:], p1[:], ACTF.Silu,
                                         scale=1.0 / SC,
                                         bias=bcq1[:, mc:mc + 1])
                    # qg' = (p2 + 256*b2) * s1 = 256*g
                    nc.vector.scalar_tensor_tensor(
                        qg[mc][:, ch, :], p2[:], bcq2x[:, mc:mc + 1], s1[:],
                        ALU.add, ALU.mult)

        # ===== stage 2b: phi_q (all-Exp) + attention tail + out-proj =====
        with ExitStack() as st2b:
            wop = st2b.enter_context(tc.tile_pool(name="wo", bufs=1))
            phiqp = st2b.enter_context(tc.tile_pool(name="phiq", bufs=3))
            attnp = st2b.enter_context(tc.tile_pool(name="attn", bufs=3))
            t2b = st2b.enter_context(tc.tile_pool(name="t2b", bufs=6))
            tzp = st2b.enter_context(tc.tile_pool(name="tz", bufs=3))
            osbp = st2b.enter_context(tc.tile_pool(name="osb", bufs=3))
            pzp = st2b.enter_context(tc.tile_pool(name="pz", bufs=2, space="PSUM"))
            prp = st2b.enter_context(tc.tile_pool(name="pr", bufs=2, space="PSUM"))
            pnp = st2b.enter_context(tc.tile_pool(name="pn", bufs=2, space="PSUM"))
            pop = st2b.enter_context(tc.tile_pool(name="po", bufs=2, space="PSUM"))

            e_sb = wop.tile([H, NCD * P], F32R, tag="eexp", name="eexp")
            nc.scalar.dma_start(e_sb[:], dt["eexp"][:].bitcast(F32R))
            wo8 = wop.tile([P, 8, D], F8, tag="wo8", name="wo8")
            nc.scalar.dma_start(wo8[:], dt["wo8p"][:])
            borrow = wop.tile([1, D], F32R, tag="borrow", name="borrow")
            nc.scalar.dma_start(borrow[:], dt["borrow"][:].bitcast(F32R))
            ones128r = wop.tile([1, P], F32R, tag="ones128r", name="ones128r")
            nc.scalar.dma_start(ones128r[:], dt["ones128r"][:].bitcast(F32R))
            epse = wop.tile([1, H], F32R, tag="epse", name="epse")
            nc.scalar.dma_start(epse[:], dt["epsrow"][:].bitcast(F32R))
            onesr = wop.tile([1, CH], F32R, tag="onesr", name="onesr")
            nc.scalar.dma_start(onesr[:], dt["onesrow"][:].bitcast(F32R))

            def phi_pass(ch):
                phi_q = [phiqp.tile([P, CH], F32R, tag=f"phiq{mc}",
                                    name=f"phiq{mc}") for mc in range(NCD)]
                for mc in range(NCD):
                    texp = t2b.tile([P, CH], F32, tag="qtexp", name="qtexp")
                    # 256*exp(g) = exp(qg'/256 + ln 256)
                    nc.scalar.activation(texp[:], qg[mc][:, ch, :], ACTF.Exp,
                                         scale=1.0 / SC, bias=lnsc[:])
                    # phi' = min(qg' + 256, 256*exp(g)) = 256*(elu(g)+1)
                    nc.vector.scalar_tensor_tensor(
                        phi_q[mc][:], qg[mc][:, ch, :], SC, texp[:],
                        ALU.add, ALU.min)
                return phi_q

            def tail_head(phi_q):
                pz = pzp.tile([H, CH], F32, tag="pz", name="pz")
                for c in range(NCD):
                    mm(pz[:], ksum_bd[c][:], phi_q[c][:],
                       start=(c == 0), stop=False)
                # eps lands in PSUM via a K=1 outer product: eps_col x ones
                mm(pz[:], epse[:], onesr[:], start=False, stop=True)
                r_sb = tzp.tile([H, CH], F32R, tag="r_sb", name="r_sb")
                with nc.allow_low_precision(reason="fp32r feeds r-broadcast mm"):
                    nc.vector.reciprocal(r_sb[:], pz[:])
                attn8 = attnp.tile([P, NCD, CH], F8, tag="attn8", name="attn8")
                return r_sb, attn8

            def tail_pair(phi_q, r_sb, attn8, pair):
                # r broadcast across each head's 64 partitions via expander E
                pr2 = prp.tile([P, CH], F32, tag="pr", name="pr")
                mm(pr2[:], e_sb[:, pair * P:(pair + 1) * P], r_sb[:],
                   start=True, stop=True)
                r_rep = t2b.tile([P, CH], F32, tag="r_rep", name="r_rep")
                if pair % 4 != 3:
                    nc.scalar.activation(r_rep[:], pr2[:], ACTF.Copy)
                else:
                    nc.vector.tensor_copy(r_rep[:], pr2[:])
                pn2 = pnp.tile([P, CH], F32, tag="pn", name="pn")
                mm(pn2[:], kv_bd[pair][:], phi_q[pair][:], start=True, stop=True)
                # attn8 = fp8(16 * num / (z+eps)) feeds the fp8 out-proj
                nc.vector.scalar_tensor_tensor(attn8[:, pair, :], pn2[:], 16.0,
                                               r_rep[:], ALU.mult, ALU.mult)

            def tail_out(ch, attn8):
                for mt in range(CH // P):
                    o_sb = osbp.tile([P, D], F32, tag="o_sb", name="o_sb")
                    for n in range(2):
                        ns = slice(n * 512, (n + 1) * 512)
                        po = pop.tile([P, 512], F32, tag="po", name="po")
                        for k in range(4):
                            mm8(po[:], attn8[:, 2 * k:2 * k + 2,
                                             mt * P:(mt + 1) * P],
                                wo8[:, 2 * k:2 * k + 2, ns],
                                start=(k == 0), stop=False)
                        # exact bias enters PSUM as a fp32r K=1 outer product
                        # (the output is the same magnitude as the bias, so
                        # the bias cannot be fp8-quantized)
                        mm(po[:], ones128r[:], borrow[:, ns],
                           start=False, stop=True)
                        nc.scalar.activation(o_sb[:, ns], po[:], ACTF.Copy,
                                             scale=1.0 / SC)
                    row0 = ch * CH + mt * P
                    nc.sync.dma_start(out[row0:row0 + P, :], o_sb[:])

            def run_tail(p_ch, p_phi):
                p_r, p_attn = tail_head(p_phi)
                for pair in range(NCD):
                    tail_pair(p_phi, p_r, p_attn, pair)
                tail_out(p_ch, p_attn)

            pending = []
            for ch in range(NCH):
                pending.append((ch, phi_pass(ch)))
                if len(pending) > 1:
                    run_tail(*pending.pop(0))
            for item in pending:
                run_tail(*item)
        qgp_ctx.close()


_CACHE = {}


def _get_nc(single_core=False):
    key = bool(single_core)
    if key not in _CACHE:
        _CACHE[key] = build(single_core)
    return _CACHE[key]


def _q8(a):
    return a.astype(ml_dtypes.float8_e4m3)


def _pack_x(xT, nt, tw):
    # xT [D, T] fp8 -> [p, tile, kt, t] -> [P, nt*8*tw]
    return np.ascontiguousarray(
        xT.reshape(8, P, nt, tw).transpose(1, 2, 0, 3).reshape(P, nt * 8 * tw))


def _pack_w(wT, b=None):
    # wT [D, D] fp32 -> fp8 [p, kt, o] (+ optional bias carrier rows kt=8,9)
    w8 = _q8(wT * 16.0).reshape(8, P, D).transpose(1, 0, 2)
    if b is None:
        return np.ascontiguousarray(w8.reshape(P, 8 * D))
    blk = np.zeros((P, 2, D), ml_dtypes.float8_e4m3)
    blk[0, 0, :] = _q8(b * 16.0)
    return np.ascontiguousarray(
        np.concatenate([w8, blk], axis=1).reshape(P, 10 * D))


def _pack_w_hilo(wT):
    # wT [D, D] fp32 -> fp8 hi (kt 0-7) + lo residual (kt 8-15), one scale
    w16 = wT.astype(np.float32) * 16.0
    hi = _q8(w16)
    lo = _q8(w16 - hi.astype(np.float32))
    pk = lambda a: a.reshape(8, P, D).transpose(1, 0, 2)
    return np.ascontiguousarray(
        np.concatenate([pk(hi), pk(lo)], axis=1).reshape(P, 16 * D))


def make_in_maps(inputs):
    f = np.float32
    q = np.asarray(inputs["query"], f).reshape(B * S, D)
    k = np.asarray(inputs["key"], f).reshape(B * S, D)
    v = np.asarray(inputs["value"], f).reshape(B * S, D)
    mask = np.asarray(inputs["mask"], f).reshape(B * S)

    carrier = np.zeros((P, 2, P), ml_dtypes.float8_e4m3)
    carrier[0, 0, :] = ml_dtypes.float8_e4m3(16.0)
    eexp = np.zeros((H, NCD * P), f)
    for pair in range(NCD):
        eexp[2 * pair, pair * P:pair * P + 64] = 1.0
        eexp[2 * pair + 1, pair * P + 64:(pair + 1) * P] = 1.0

    shared = {
        "wk18p": _pack_w(np.asarray(inputs["k_w1"], f).T,
                         np.asarray(inputs["k_b1"], f)),
        "wk28p": _pack_w(np.asarray(inputs["k_w2"], f).T,
                         np.asarray(inputs["k_b2"], f)),
        "wv18p": _pack_w(np.asarray(inputs["v_w1"], f).T,
                         np.asarray(inputs["v_b1"], f)),
        "wv28p": _pack_w(np.asarray(inputs["v_w2"], f).T,
                         np.asarray(inputs["v_b2"], f)),
        "wq18p": _pack_w(np.asarray(inputs["q_w1"], f).T),
        "wq28p": _pack_w(np.asarray(inputs["q_w2"], f).T),
        "wo8p": _pack_w(np.asarray(inputs["out_w"], f).T),
        "borrow": np.asarray(inputs["out_b"], f).reshape(1, D) * SC,
        "ones128r": np.ones((1, P), f),
        "bcq1": np.ascontiguousarray(np.asarray(inputs["q_b1"], f).reshape(NCD, P).T),
        "bcq2x": np.ascontiguousarray(
            (np.asarray(inputs["q_b2"], f) * SC).reshape(NCD, P).T),
        "zeros128": np.zeros((P, P), f),
        "zeros512": np.zeros((P, 4 * P), f),
        "epsrow": np.full((1, H), SC * EPS, f),
        "onesrow": np.ones((1, CH), f),
        "carrier": np.ascontiguousarray(carrier.reshape(P, 2 * P)),
        "eexp": eexp,
    }
    in_maps = []
    for c in range(NCORES):
        sl = slice(c * T, (c + 1) * T)
        m = dict(shared)
        m["xq8p"] = _pack_x(_q8(q[sl].T * 16.0), NCH, CH)
        m["xk8p"] = _pack_x(_q8(k[sl].T * 16.0), NM, P)
        m["xv8p"] = _pack_x(_q8(v[sl].T * 16.0), NM, P)
        mcol = np.ascontiguousarray(mask[sl].reshape(NM, P).T)
        m["maskb"] = mcol.astype(ml_dtypes.bfloat16)
        m["maskv"] = np.ascontiguousarray(mcol / SC)
        in_maps.append(m)
    return in_maps


def kernel(**inputs):
    nc = _get_nc(False)
    in_maps = make_in_maps(inputs)
    res = run_bass_kernel_spmd(nc, in_maps, list(range(NCORES))).results
    outc = np.concatenate([res[c]["out"] for c in range(NCORES)], axis=0)
    return outc.reshape(B, S, D)


# revision 104
# speedup vs baseline: 1.0078x; 1.0016x over previous
"""MultiHeadLinearAttention Trainium2 kernel (8-core SPMD, fp8 DoubleRow).

Sharding: 16384 tokens split across 8 cores (core c: batch c//2, sequence half
c%2). All projections/attention/out-proj are local; the only cross-core
dependency is the per-batch KV summary (kv [H,DK,DK] + ksum [D]) reduced via a
266KB pair-wise AllReduce, overlapped with stage-2a q GLU compute.

Numerics: all seven big GEMMs (q/k/v GLU pairs + out-proj) run in fp8 e4m3
DoubleRow mode (host quantizes x*16 and W*16; PSUM carries 256*y with fp32
accumulation; attn is cast to fp8*16 on its PSUM eviction). DoubleRow
contracts two 128-row k-tiles per instruction at 0.5 cycles/row -- 4x fp32r
throughput. The attention summary (kv, ksum, z, num) stays bf16/fp32r.
End-to-end rel err vs the fp32 reference: 1.47e-2 (harness gate 2e-2).
The out-proj BIAS must stay exact (the output is the same magnitude as the
bias), so it enters PSUM as a fp32r K=1 outer product, not fp8.

Elementwise: GLU = 1 ACT Silu + 1 DVE stt (k/v bias enters PSUM via a
constant fp8 carrier k-tile pair; q bias via ACT per-partition bias/scale).
phi = elu(x)+1 computed exactly as min(exp(x), x+1): 1 ACT Exp + 1 DVE stt
(Exp overflow to +inf is absorbed by the min). The mask multiplies are folded
away: ksum takes the mask column as its matmul lhsT and vg's stt scalar slot
carries mask/256.

ACT table discipline: silu and exp live in different ACT table sets (1283ns
per reload), so stages are ordered k-GLU, v-GLU (all Silu), phi_k/ksum/kv
pass (all Exp), then stage 2 where the scheduler interleaves q-GLU (Silu)
chunks with the lagged phi_q/attention tail (Exp) at 2 reloads per chunk.

The reciprocal 1/(z+eps) is broadcast across each head's 64 partitions by one
matmul per head pair against a constant block "expander" E [16,128] (eps is
accumulated into the z PSUM as a K=1 outer product); num for both heads of a
pair comes from one full-array matmul against a block-diagonal kv tile.
"""
import os
from contextlib import ExitStack

import ml_dtypes
import numpy as np
import concourse.mybir as mybir
import concourse.tile as tile
from concourse import bacc
from concourse.bass_utils import run_bass_kernel_spmd

F32 = mybir.dt.float32
F32R = mybir.dt.float32r
F8 = mybir.dt.float8e4
BF16 = mybir.dt.bfloat16
ACTF = mybir.ActivationFunctionType
ALU = mybir.AluOpType
DR = mybir.MatmulPerfMode.DoubleRow

B, S, D, H = 4, 4096, 1024, 16
DK = D // H          # 64
EPS = 1e-6
NCORES = 8
T = B * S // NCORES  # 2048 tokens per core
P = 128
NM = T // P          # 16 token tiles
NCD = D // P         # 8 d-chunks
CH = 512             # stage-2 token chunk
NCH = T // CH        # 8 chunks
GROUPS = [[0, 1], [2, 3], [4, 5], [6, 7]]
SC = 256.0           # fp8 GEMM psum scale (sx*sw = 16*16)
LN_SC = float(np.log(SC))


def build(single_core=False):
    nc = bacc.Bacc("TRN2", target_bir_lowering=False, debug=False,
                   num_devices=1 if single_core else NCORES)
    dt_in = {}

    def inp(name, shape, dty=F32):
        dt_in[name] = nc.dram_tensor(name, shape, dty, kind="ExternalInput").ap()

    for name, shape in (
        ("xk8p", [P, NM * 8 * P]), ("xv8p", [P, NM * 8 * P]),
        ("xq8p", [P, NCH * 8 * CH]),
        ("wk18p", [P, 10 * D]), ("wk28p", [P, 10 * D]),
        ("wv18p", [P, 10 * D]), ("wv28p", [P, 10 * D]),
        ("wq18p", [P, 8 * D]), ("wq28p", [P, 8 * D]),
        ("wo8p", [P, 8 * D]), ("carrier", [P, 2 * P]),
    ):
        inp(name, shape, F8)
    for name, shape in (
        ("maskv", [P, NM]), ("bcq1", [P, NCD]), ("bcq2x", [P, NCD]),
        ("epsrow", [1, H]), ("onesrow", [1, CH]),
        ("zeros128", [P, P]), ("zeros512", [P, 4 * P]),
        ("eexp", [H, NCD * P]), ("borrow", [1, D]), ("ones128r", [1, P]),
    ):
        inp(name, shape, F32)
    inp("maskb", [P, NM], BF16)
    out = nc.dram_tensor("out", [T, D], F32, kind="ExternalOutput").ap()

    with tile.TileContext(nc) as tc:
        _emit(nc, tc, dt_in, out, single_core)
    nc.compile()
    return nc


def _emit(nc, tc, dt, out, single_core):
    def mm(psum, lhsT, rhs, start, stop):
        nc.tensor.matmul(psum, lhsT, rhs, start=start, stop=stop)

    def mm8(psum, lhsT, rhs, start, stop):
        nc.tensor.matmul(psum, lhsT, rhs, start=start, stop=stop, perf_mode=DR)

    with ExitStack() as st0:
        const = st0.enter_context(tc.tile_pool(name="const", bufs=1))
        dram = st0.enter_context(tc.tile_pool(name="dram", bufs=1, space="DRAM"))
        kvres = st0.enter_context(tc.tile_pool(name="kvres", bufs=1))
        kvstage_ctx = ExitStack()
        kvstage = kvstage_ctx.enter_context(tc.tile_pool(name="kvstage", bufs=1))

        carrier = const.tile([P, 2, P], F8, tag="carrier", name="carrier")
        nc.gpsimd.dma_start(carrier[:], dt["carrier"][:])
        maskb = const.tile([P, NM], BF16, tag="maskb", name="maskb")
        nc.gpsimd.dma_start(maskb[:], dt["maskb"][:])
        maskv = const.tile([P, NM], F32, tag="maskv", name="maskv")
        nc.gpsimd.dma_start(maskv[:], dt["maskv"][:])
        lnsc = const.tile([P, 1], F32, tag="lnsc", name="lnsc")
        nc.any.memset(lnsc[:], LN_SC)

        # ---- stage A/B/C pools (whole-stage-resident tiles) ----
        stab = st0.enter_context(ExitStack())
        kgp = stab.enter_context(tc.tile_pool(name="kgp", bufs=1))
        vgp = stab.enter_context(tc.tile_pool(name="vgp", bufs=1))
        phikp_ctx = ExitStack()
        phikp = phikp_ctx.enter_context(tc.tile_pool(name="phik", bufs=1))
        kg = [kgp.tile([P, D], BF16, tag=f"kg{m}", name=f"kg{m}")
              for m in range(NM)]
        vg = [vgp.tile([P, D], BF16, tag=f"vg{m}", name=f"vg{m}")
              for m in range(NM)]
        phi_k = [phikp.tile([P, D], BF16, tag=f"phik{m}", name=f"phik{m}")
                 for m in range(NM)]

        stxw = ExitStack()
        wkvp = stxw.enter_context(tc.tile_pool(name="wkv", bufs=1))
        xkvp = stxw.enter_context(tc.tile_pool(name="xkv", bufs=1))
        # startup order: first k-tile x quarter + first weight halves land
        # before the rest so m=0 GEMMs start ~5us in
        x_sb = {}
        x_sb["k"] = xkvp.tile([P, NM, 8, P], F8, tag="xk", name="xk")
        nc.sync.dma_start(x_sb["k"][:, 0:4, :, :], dt["xk8p"][:, 0:4096])
        w_sb = {}
        for w, src in (("k1", "wk18p"), ("k2", "wk28p")):
            w_sb[w] = wkvp.tile([P, 10, D], F8, tag=f"w{w}", name=f"w{w}")
            nc.sync.dma_start(
                w_sb[w][:, :, 0:512],
                dt[src][:].rearrange("p (kt o) -> p kt o", kt=10)[:, :, 0:512])
        for w, src in (("k1", "wk18p"), ("k2", "wk28p")):
            nc.sync.dma_start(
                w_sb[w][:, :, 512:D],
                dt[src][:].rearrange("p (kt o) -> p kt o", kt=10)[:, :, 512:D])
        for qtr in range(1, 4):
            nc.sync.dma_start(x_sb["k"][:, qtr * 4:(qtr + 1) * 4, :, :],
                              dt["xk8p"][:, qtr * 4096:(qtr + 1) * 4096])
        # prefetch v inputs (overlaps stage A compute)
        for w, src in (("v1", "wv18p"), ("v2", "wv28p")):
            w_sb[w] = wkvp.tile([P, 10, D], F8, tag=f"w{w}", name=f"w{w}")
            nc.gpsimd.dma_start(w_sb[w][:], dt[src][:])
        x_sb["v"] = xkvp.tile([P, NM, 8, P], F8, tag="xv", name="xv")
        for qtr in range(4):
            nc.gpsimd.dma_start(x_sb["v"][:, qtr * 4:(qtr + 1) * 4, :, :],
                                dt["xv8p"][:, qtr * 4096:(qtr + 1) * 4096])

        # ============ stages A (k GLU) and B (v GLU): all-Silu ============
        def glu_stage(which, out_tiles):
            with ExitStack() as stg:
                p1p = stg.enter_context(
                    tc.tile_pool(name=f"p1{which}", bufs=4, space="PSUM"))
                p2p = stg.enter_context(
                    tc.tile_pool(name=f"p2{which}", bufs=4, space="PSUM"))
                tp = stg.enter_context(tc.tile_pool(name=f"t{which}", bufs=6))
                w1 = w_sb[which + "1"]
                w2 = w_sb[which + "2"]
                xt = x_sb[which]
                for m in range(NM):
                    for n in range(2):
                        ns = slice(n * 512, (n + 1) * 512)
                        p1 = p1p.tile([P, 512], F32, tag="p1", name="p1")
                        p2 = p2p.tile([P, 512], F32, tag="p2", name="p2")
                        for k in range(4):
                            mm8(p1[:], xt[:, m, 2 * k:2 * k + 2, :],
                                w1[:, 2 * k:2 * k + 2, ns],
                                start=(k == 0), stop=False)
                        mm8(p1[:], carrier[:], w1[:, 8:10, ns],
                            start=False, stop=True)
                        for k in range(4):
                            mm8(p2[:], xt[:, m, 2 * k:2 * k + 2, :],
                                w2[:, 2 * k:2 * k + 2, ns],
                                start=(k == 0), stop=False)
                        mm8(p2[:], carrier[:], w2[:, 8:10, ns],
                            start=False, stop=True)
                        s1 = tp.tile([P, 512], F32, tag="s1", name="s1")
                        nc.scalar.activation(s1[:], p1[:], ACTF.Silu,
                                             scale=1.0 / SC)
                        if which == "k":
                            # kg = (p2/SC) * s1  (true scale)
                            nc.vector.scalar_tensor_tensor(
                                out_tiles[m][:, ns], p2[:], 1.0 / SC, s1[:],
                                ALU.mult, ALU.mult)
                        else:
                            # vg = (p2 * mask/SC) * s1  (mask folded in)
                            nc.vector.scalar_tensor_tensor(
                                out_tiles[m][:, ns], p2[:], maskv[:, m:m + 1],
                                s1[:], ALU.mult, ALU.mult)

        glu_stage("k", kg)
        glu_stage("v", vg)
        stxw.close()  # frees k/v weights + x tiles before stage-2 prefetch

        # prefetch stage-2 inputs (overlap stage C + collective)
        st2pre = st0.enter_context(ExitStack())
        wqp = st2pre.enter_context(tc.tile_pool(name="wqp", bufs=1, side="right"))
        wq1 = wqp.tile([P, 8, D], F8, tag="wq1", name="wq1")
        nc.sync.dma_start(wq1[:], dt["wq18p"][:])
        wq2 = wqp.tile([P, 8, D], F8, tag="wq2", name="wq2")
        nc.sync.dma_start(wq2[:], dt["wq28p"][:])
        xq8 = wqp.tile([P, NCH, 8, CH], F8, tag="xq8", name="xq8")
        for qtr in range(NCH):
            nc.sync.dma_start(
                xq8[:, qtr, :, :],
                dt["xq8p"][:, qtr * 8 * CH:(qtr + 1) * 8 * CH])

        # ====== stage C: phi_k (all-Exp) + ksum + kv accumulation ======
        with ExitStack() as stc:
            tcp = stc.enter_context(tc.tile_pool(name="tc", bufs=6))
            pksp = stc.enter_context(tc.tile_pool(name="pks", bufs=1, space="PSUM"))
            pkvp = stc.enter_context(tc.tile_pool(name="pkv", bufs=1, space="PSUM"))
            psum_ks = [pksp.tile([1, 512], F32, tag=f"ks{i}", name=f"ks{i}")
                       for i in range(2)]
            psum_kv = [pkvp.tile([64, 512], F32, tag=f"pkv{i}", name=f"pkv{i}")
                       for i in range(2)]

            def ksum_tail(m):
                for i in range(2):
                    mm(psum_ks[i][:], maskb[:, m:m + 1],
                       phi_k[m][:, i * 512:(i + 1) * 512],
                       start=(m == 0), stop=(m == NM - 1))

            def kv_tail(m):
                # one global accumulation group per bank: start only on the
                # very first matmul (has_written is per element)
                for h in range(H):
                    hs = slice(h * DK, (h + 1) * DK)
                    first = (m == 0 and h % 8 == 0)
                    last = (m == NM - 1 and h % 8 == 7)
                    nc.tensor.matmul(
                        psum_kv[h // 8][0:64, (h % 8) * DK:(h % 8 + 1) * DK],
                        phi_k[m][:, hs], vg[m][:, hs],
                        start=first, stop=last,
                        skip_group_check=not (first or last))

            for m in range(NM):
                texp = tcp.tile([P, D], F32, tag="texp", name="texp")
                nc.scalar.activation(texp[:], kg[m][:], ACTF.Exp)
                # phi_k = min(kg + 1, exp(kg)) = elu(kg) + 1
                nc.vector.scalar_tensor_tensor(
                    phi_k[m][:], kg[m][:], 1.0, texp[:], ALU.add, ALU.min)
                if m >= 2:
                    ksum_tail(m - 2)
                    kv_tail(m - 2)
            for m in (NM - 2, NM - 1):
                ksum_tail(m)
                kv_tail(m)
            cc_ks_sb = kvstage.tile([1, D], F32, tag="cc_ks_sb", name="cc_ks_sb")
            for i in range(2):
                nc.vector.tensor_copy(cc_ks_sb[0:1, i * 512:(i + 1) * 512],
                                      psum_ks[i][:])
            kv_acc = [kvstage.tile([64, 512], F32, tag=f"kv_acc{i}",
                                   name=f"kv_acc{i}") for i in range(2)]
            for i in range(2):
                nc.vector.tensor_copy(kv_acc[i][:], psum_kv[i][:])

        phikp_ctx.close()
        stab.close()     # frees kg/vg tiles

        # ============ collective: pair AllReduce of kv + ksum ============
        cc_in = dram.tile([130, 512], F32)
        cc_out = dram.tile([130, 512], F32)
        nc.gpsimd.dma_start(cc_in[0:64, :], kv_acc[0][:])
        nc.gpsimd.dma_start(cc_in[64:128, :], kv_acc[1][:])
        nc.gpsimd.dma_start(cc_in[128:130, :], cc_ks_sb[:])
        kvstage_ctx.close()
        if single_core:
            nc.sync.dma_start(cc_out[:], cc_in[:])
        else:
            nc.gpsimd.collective_compute(
                "AllReduce", ALU.add, replica_groups=GROUPS,
                ins=[cc_in.opt()], outs=[cc_out.opt()])

        # reduced kv -> per-pair block-diag lhsT slices of one tile (a single
        # full-array matmul computes both heads of a pair; avoids PE
        # quadrant-3 subtiling). All rebuild DMAs go on the idle SP engine.
        kv_bd_t = kvres.tile([P, NCD, P], F32R, tag="kvbd", name="kvbd")
        for i in range(2):
            nc.sync.dma_start(kv_bd_t[:, 4 * i:4 * i + 4, :],
                              dt["zeros512"][:].bitcast(F32R))
        for pair in range(NCD):
            for i, h in ((0, 2 * pair), (64, 2 * pair + 1)):
                r0 = 0 if h < 8 else 64
                nc.sync.dma_start(
                    kv_bd_t[i:i + 64, pair, i:i + 64],
                    cc_out[r0:r0 + 64,
                           (h % 8) * DK:(h % 8 + 1) * DK].bitcast(F32R))
        kv_bd = [kv_bd_t[:, pair, :] for pair in range(NCD)]
        ksum_bd_t = kvres.tile([P, NCD, H], F32R, tag="ksbd", name="ksbd")
        nc.sync.dma_start(ksum_bd_t[:, :, :],
                          dt["zeros128"][:, 0:NCD * H].bitcast(F32R))
        for c in range(NCD):
            # ksum[d] lives at cc_out[128 + d // 512, d % 512]
            for half, cs in ((0, 2 * c), (64, 2 * c + 1)):
                d0 = c * P + half
                nc.sync.dma_start(
                    ksum_bd_t[half:half + 64, c, cs:cs + 1],
                    cc_out[128 + d0 // 512:129 + d0 // 512,
                           d0 % 512:d0 % 512 + 64].bitcast(F32R))
        ksum_bd = [ksum_bd_t[:, c, :] for c in range(NCD)]

        # ============ stage 2a: q GLU (all-Silu) -> qg (256x scale) ============
        bcq1 = const.tile([P, NCD], F32, tag="bcq1", name="bcq1")
        nc.gpsimd.dma_start(bcq1[:], dt["bcq1"][:])
        bcq2x = const.tile([P, NCD], F32, tag="bcq2x", name="bcq2x")
        nc.gpsimd.dma_start(bcq2x[:], dt["bcq2x"][:])

        qgp_ctx = ExitStack()
        qgp = qgp_ctx.enter_context(tc.tile_pool(name="qgp", bufs=1))
        qg = [qgp.tile([P, NCH, CH], BF16, tag=f"qg{mc}", name=f"qg{mc}")
              for mc in range(NCD)]
        with ExitStack() as st2a:
            pq1p = st2a.enter_context(tc.tile_pool(name="pq1", bufs=4, space="PSUM"))
            pq2p = st2a.enter_context(tc.tile_pool(name="pq2", bufs=4, space="PSUM"))
            t2a = st2a.enter_context(tc.tile_pool(name="t2a", bufs=6))
            for ch in range(NCH):
                for mc in range(NCD):
                    ms = slice(mc * P, (mc + 1) * P)
                    p1 = pq1p.tile([P, CH], F32, tag="pq1", name="pq1")
                    p2 = pq2p.tile([P, CH], F32, tag="pq2", name="pq2")
                    for k in range(4):
                        mm8(p1[:], wq1[:, 2 * k:2 * k + 2, ms],
                            xq8[:, ch, 2 * k:2 * k + 2, :],
                            start=(k == 0), stop=(k == 3))
                    for k in range(4):
                        mm8(p2[:], wq2[:, 2 * k:2 * k + 2, ms],
                            xq8[:, ch, 2 * k:2 * k + 2, :],
                            start=(k == 0), stop=(k == 3))
                    s1 = t2a.tile([P, CH], F32, tag="qs1", name="qs1")
                    nc.scalar.activation(s1[:], p1[:], ACTF.Silu,
                                         scale=1.0 / SC,
                                         bias=bcq1[:, mc:mc + 1])
                    # qg' = (p2 + 256*b2) * s1 = 256*g
                    nc.vector.scalar_tensor_tensor(
                        qg[mc][:, ch, :], p2[:], bcq2x[:, mc:mc + 1], s1[:],
                        ALU.add, ALU.mult)

        # ===== stage 2b: phi_q (all-Exp) + attention tail + out-proj =====
        with ExitStack() as st2b:
            wop = st2b.enter_context(tc.tile_pool(name="wo", bufs=1))
            phiqp = st2b.enter_context(tc.tile_pool(name="phiq", bufs=3))
            attnp = st2b.enter_context(tc.tile_pool(name="attn", bufs=4))
            t2b = st2b.enter_context(tc.tile_pool(name="t2b", bufs=6))
            tzp = st2b.enter_context(tc.tile_pool(name="tz", bufs=6))
            osbp = st2b.enter_context(tc.tile_pool(name="osb", bufs=5))
            pzp = st2b.enter_context(tc.tile_pool(name="pz", bufs=2, space="PSUM"))
            prp = st2b.enter_context(tc.tile_pool(name="pr", bufs=2, space="PSUM"))
            pnp = st2b.enter_context(tc.tile_pool(name="pn", bufs=2, space="PSUM"))
            pop = st2b.enter_context(tc.tile_pool(name="po", bufs=2, space="PSUM"))

            e_sb = wop.tile([H, NCD * P], F32R, tag="eexp", name="eexp")
            nc.scalar.dma_start(e_sb[:], dt["eexp"][:].bitcast(F32R))
            wo8 = wop.tile([P, 8, D], F8, tag="wo8", name="wo8")
            nc.scalar.dma_start(wo8[:], dt["wo8p"][:])
            borrow = wop.tile([1, D], F32R, tag="borrow", name="borrow")
            nc.scalar.dma_start(borrow[:], dt["borrow"][:].bitcast(F32R))
            ones128r = wop.tile([1, P], F32R, tag="ones128r", name="ones128r")
            nc.scalar.dma_start(ones128r[:], dt["ones128r"][:].bitcast(F32R))
            epse = wop.tile([1, H], F32R, tag="epse", name="epse")
            nc.scalar.dma_start(epse[:], dt["epsrow"][:].bitcast(F32R))
            onesr = wop.tile([1, CH], F32R, tag="onesr", name="onesr")
            nc.scalar.dma_start(onesr[:], dt["onesrow"][:].bitcast(F32R))

            def phi_pass(ch):
                phi_q = [phiqp.tile([P, CH], F32R, tag=f"phiq{mc}",
                                    name=f"phiq{mc}") for mc in range(NCD)]
                for mc in range(NCD):
                    texp = t2b.tile([P, CH], F32, tag="qtexp", name="qtexp")
                    # 256*exp(g) = exp(qg'/256 + ln 256)
                    nc.scalar.activation(texp[:], qg[mc][:, ch, :], ACTF.Exp,
                                         scale=1.0 / SC, bias=lnsc[:])
                    # phi' = min(qg' + 256, 256*exp(g)) = 256*(elu(g)+1)
                    nc.vector.scalar_tensor_tensor(
                        phi_q[mc][:], qg[mc][:, ch, :], SC, texp[:],
                        ALU.add, ALU.min)
                return phi_q

            def tail_head(phi_q):
                pz = pzp.tile([H, CH], F32, tag="pz", name="pz")
                for c in range(NCD):
                    mm(pz[:], ksum_bd[c][:], phi_q[c][:],
                       start=(c == 0), stop=False)
                # eps lands in PSUM via a K=1 outer product: eps_col x ones
                mm(pz[:], epse[:], onesr[:], start=False, stop=True)
                r_sb = tzp.tile([H, CH], F32R, tag="r_sb", name="r_sb")
                with nc.allow_low_precision(reason="fp32r feeds r-broadcast mm"):
                    nc.vector.reciprocal(r_sb[:], pz[:])
                attn8 = attnp.tile([P, NCD, CH], F8, tag="attn8", name="attn8")
                return r_sb, attn8

            def tail_pair(phi_q, r_sb, attn8, pair):
                # r broadcast across each head's 64 partitions via expander E
                pr2 = prp.tile([P, CH], F32, tag="pr", name="pr")
                mm(pr2[:], e_sb[:, pair * P:(pair + 1) * P], r_sb[:],
                   start=True, stop=True)
                r_rep = t2b.tile([P, CH], F32, tag="r_rep", name="r_rep")
                if pair % 4 != 3:
                    nc.scalar.activation(r_rep[:], pr2[:], ACTF.Copy)
                else:
                    nc.vector.tensor_copy(r_rep[:], pr2[:])
                pn2 = pnp.tile([P, CH], F32, tag="pn", name="pn")
                mm(pn2[:], kv_bd[pair][:], phi_q[pair][:], start=True, stop=True)
                # attn8 = fp8(16 * num / (z+eps)) feeds the fp8 out-proj
                nc.vector.scalar_tensor_tensor(attn8[:, pair, :], pn2[:], 16.0,
                                               r_rep[:], ALU.mult, ALU.mult)

            def tail_out(ch, attn8):
                for mt in range(CH // P):
                    o_sb = osbp.tile([P, D], F32, tag="o_sb", name="o_sb")
                    for n in range(2):
                        ns = slice(n * 512, (n + 1) * 512)
                        po = pop.tile([P, 512], F32, tag="po", name="po")
                        for k in range(4):
                            mm8(po[:], attn8[:, 2 * k:2 * k + 2,
                                             mt * P:(mt + 1) * P],
                                wo8[:, 2 * k:2 * k + 2, ns],
                                start=(k == 0), stop=False)
                        # exact bias enters PSUM as a fp32r K=1 outer product
                        # (the output is the same magnitude as the bias, so
                        # the bias cannot be fp8-quantized)
                        mm(po[:], ones128r[:], borrow[:, ns],
                           start=False, stop=True)
                        nc.scalar.activation(o_sb[:, ns], po[:], ACTF.Copy,
                                             scale=1.0 / SC)
                    row0 = ch * CH + mt * P
                    nc.sync.dma_start(out[row0:row0 + P, :], o_sb[:])

            def run_tail(p_ch, p_phi):
                p_r, p_attn = tail_head(p_phi)
                for pair in range(NCD):
                    tail_pair(p_phi, p_r, p_attn, pair)
                tail_out(p_ch, p_attn)

            pending = []
            for ch in range(NCH):
                pending.append((ch, phi_pass(ch)))
                if len(pending) > 1:
                    run_tail(*pending.pop(0))
            for item in pending:
                run_tail(*item)
        qgp_ctx.close()


_CACHE = {}


def _get_nc(single_core=False):
    key = bool(single_core)
    if key not in _CACHE:
        _CACHE[key] = build(single_core)
    return _CACHE[key]


def _q8(a):
    return a.astype(ml_dtypes.float8_e4m3)


def _pack_x(xT, nt, tw):
    # xT [D, T] fp8 -> [p, tile, kt, t] -> [P, nt*8*tw]
    return np.ascontiguousarray(
        xT.reshape(8, P, nt, tw).transpose(1, 2, 0, 3).reshape(P, nt * 8 * tw))


def _pack_w(wT, b=None):
    # wT [D, D] fp32 -> fp8 [p, kt, o] (+ optional bias carrier rows kt=8,9)
    w8 = _q8(wT * 16.0).reshape(8, P, D).transpose(1, 0, 2)
    if b is None:
        return np.ascontiguousarray(w8.reshape(P, 8 * D))
    blk = np.zeros((P, 2, D), ml_dtypes.float8_e4m3)
    blk[0, 0, :] = _q8(b * 16.0)
    return np.ascontiguousarray(
        np.concatenate([w8, blk], axis=1).reshape(P, 10 * D))


def _pack_w_hilo(wT):
    # wT [D, D] fp32 -> fp8 hi (kt 0-7) + lo residual (kt 8-15), one scale
    w16 = wT.astype(np.float32) * 16.0
    hi = _q8(w16)
    lo = _q8(w16 - hi.astype(np.float32))
    pk = lambda a: a.reshape(8, P, D).transpose(1, 0, 2)
    return np.ascontiguousarray(
        np.concatenate([pk(hi), pk(lo)], axis=1).reshape(P, 16 * D))


def make_in_maps(inputs):
    f = np.float32
    q = np.asarray(inputs["query"], f).reshape(B * S, D)
    k = np.asarray(inputs["key"], f).reshape(B * S, D)
    v = np.asarray(inputs["value"], f).reshape(B * S, D)
    mask = np.asarray(inputs["mask"], f).reshape(B * S)

    carrier = np.zeros((P, 2, P), ml_dtypes.float8_e4m3)
    carrier[0, 0, :] = ml_dtypes.float8_e4m3(16.0)
    eexp = np.zeros((H, NCD * P), f)
    for pair in range(NCD):
        eexp[2 * pair, pair * P:pair * P + 64] = 1.0
        eexp[2 * pair + 1, pair * P + 64:(pair + 1) * P] = 1.0

    shared = {
        "wk18p": _pack_w(np.asarray(inputs["k_w1"], f).T,
                         np.asarray(inputs["k_b1"], f)),
        "wk28p": _pack_w(np.asarray(inputs["k_w2"], f).T,
                         np.asarray(inputs["k_b2"], f)),
        "wv18p": _pack_w(np.asarray(inputs["v_w1"], f).T,
                         np.asarray(inputs["v_b1"], f)),
        "wv28p": _pack_w(np.asarray(inputs["v_w2"], f).T,
                         np.asarray(inputs["v_b2"], f)),
        "wq18p": _pack_w(np.asarray(inputs["q_w1"], f).T),
        "wq28p": _pack_w(np.asarray(inputs["q_w2"], f).T),
        "wo8p": _pack_w(np.asarray(inputs["out_w"], f).T),
        "borrow": np.asarray(inputs["out_b"], f).reshape(1, D) * SC,
        "ones128r": np.ones((1, P), f),
        "bcq1": np.ascontiguousarray(np.asarray(inputs["q_b1"], f).reshape(NCD, P).T),
        "bcq2x": np.ascontiguousarray(
            (np.asarray(inputs["q_b2"], f) * SC).reshape(NCD, P).T),
        "zeros128": np.zeros((P, P), f),
        "zeros512": np.zeros((P, 4 * P), f),
        "epsrow": np.full((1, H), SC * EPS, f),
        "onesrow": np.ones((1, CH), f),
        "carrier": np.ascontiguousarray(carrier.reshape(P, 2 * P)),
        "eexp": eexp,
    }
    in_maps = []
    for c in range(NCORES):
        sl = slice(c * T, (c + 1) * T)
        m = dict(shared)
        m["xq8p"] = _pack_x(_q8(q[sl].T * 16.0), NCH, CH)
        m["xk8p"] = _pack_x(_q8(k[sl].T * 16.0), NM, P)
        m["xv8p"] = _pack_x(_q8(v[sl].T * 16.0), NM, P)
        mcol = np.ascontiguousarray(mask[sl].reshape(NM, P).T)
        m["maskb"] = mcol.astype(ml_dtypes.bfloat16)
        m["maskv"] = np.ascontiguousarray(mcol / SC)
        in_maps.append(m)
    return in_maps


def kernel(**inputs):
    nc = _get_nc(False)
    in_maps = make_in_maps(inputs)
    res = run_bass_kernel_spmd(nc, in_maps, list(range(NCORES))).results
    outc = np.concatenate([res[c]["out"] for c in range(NCORES)], axis=0)
    return outc.reshape(B, S, D)
